# revision 10
# baseline (speedup 1.0000x reference)
"""ChannelKiller kernel for Trainium2 (8 NeuronCores, SPMD).

Computes out[b, c, t] = x[b, c, t] * (1.0 if c == 0 else 0.5) for
x of shape (16, 8, 262144) f32.

Strategy: pure elementwise, memory-bound (per-core HBM roofline ~94 us for
16 MiB in + 16 MiB out at ~358 GB/s). Shard along batch: core i gets
x[2i:2i+2]. Each per-core batch (8, 262144) is viewed as [128, 16384]
(partition p covers channel p//16), so the channel scale is per-partition.
Pipeline per tile: DMA load -> mul 0.5 (all partitions) -> mul 2.0 on
partitions 0..15 to restore channel 0 (compute APs need quadrant-aligned
partition bases, so a direct 0.5x on partitions 16..127 is illegal; the
0.5*2 round trip is exact in fp32 for normal values) -> DMA store; load/
compute/store overlap via the Tile framework (measured ~95 us/core
steady-state, ~98% of roofline).
"""

import numpy as np

import concourse.bacc as bacc
import concourse.mybir as mybir
from concourse.bass_utils import run_bass_kernel_spmd
from concourse.tile import TileContext

N_CORES = 8
B, C, T = 16, 8, 262144
B_LOC = B // N_CORES            # batches per core = 2
P = 128                         # SBUF partitions
ROWS_PER_BATCH = C * T // P     # free elems per partition per batch = 16384
P_PER_C = P // C                # partitions per channel = 16
TILE_F = 8192                   # free-dim tile size (32 KiB/partition, 4 MiB/tile)
BUFS = 4

_NC_CACHE = None


def _build():
    global _NC_CACHE
    if _NC_CACHE is not None:
        return _NC_CACHE
    nc = bacc.Bacc("TRN2", target_bir_lowering=False, debug=False, num_devices=N_CORES)
    x = nc.declare_dram_parameter(
        "x", [B_LOC, P, ROWS_PER_BATCH], mybir.dt.float32, isOutput=False
    )
    out = nc.declare_dram_parameter(
        "out", [B_LOC, P, ROWS_PER_BATCH], mybir.dt.float32, isOutput=True
    )
    with TileContext(nc) as tc:
        with tc.tile_pool(name="io", bufs=BUFS) as pool:
            for b in range(B_LOC):
                xb = x[b]
                ob = out[b]
                for t0 in range(0, ROWS_PER_BATCH, TILE_F):
                    tile = pool.tile([P, TILE_F], mybir.dt.float32)
                    nc.sync.dma_start(out=tile[:, :], in_=xb[:, t0 : t0 + TILE_F])
                    # Compute-engine APs need quadrant-aligned partition bases,
                    # so scale everything by 0.5 then restore channel 0
                    # (partitions 0..15) with x2 — exact in fp32 for normals.
                    nc.vector.tensor_scalar_mul(tile[:, :], tile[:, :], 0.5)
                    nc.vector.tensor_scalar_mul(
                        tile[:P_PER_C, :], tile[:P_PER_C, :], 2.0
                    )
                    nc.sync.dma_start(out=ob[:, t0 : t0 + TILE_F], in_=tile[:, :])
    nc.finalize()
    _NC_CACHE = nc
    return nc


def kernel(x: np.ndarray) -> np.ndarray:
    x = np.ascontiguousarray(np.asarray(x, dtype=np.float32))
    assert x.shape == (B, C, T), x.shape
    nc = _build()

    shards = x.reshape(N_CORES, B_LOC, P, ROWS_PER_BATCH)
    in_maps = [{"x": shards[i]} for i in range(N_CORES)]
    r = run_bass_kernel_spmd(nc, in_maps, list(range(N_CORES)))

    out = np.concatenate(
        [r.results[i]["out"].reshape(B_LOC, C, T) for i in range(N_CORES)], axis=0
    )
    return out
